# revision 34
# baseline (speedup 1.0000x reference)
"""LIF neuron scan kernel for Trainium2, sharded over 8 NeuronCores.

Reference semantics (per element, T=16 steps):
    mem = mem / 5.0 + x_t
    spike = (mem - 0.5) > 0
    mem = (1 - spike) * mem

Sharding: batch dim B=64 -> 8 batches per core, no cross-core
communication. Each core's shard is transposed on host to t-major
[T, BC*N] contiguous fp16 so every timestep slice is one [128, 4096]
fp16 tile (half the HBM traffic of f32).

The whole membrane update is ONE custom DVE op per timestep
(registered at import via the documented dve_ops.OPS mechanism; the
uop program is written into the per-NEFF DVE table at compile time,
no firmware change):

    m_t = select(m_{t-1} <= 0.5, m_{t-1} * 0.2, 0) + x_t

replacing the previous 3-op DVE chain (TT add 2x + TS 4x + TT mult 2x,
5515 ns engine time/step) with a single 1x fused op. The DVE computes
the body in f32 internally and rounds once to the fp16 carry
(HW-verified bit-identical to the host numpy model: 5711 spike flips
of 67.1M vs the f32 oracle, rel err 1.729e-2 < 2e-2 gate; an f32
carry gives 3533 flips/1.36e-2 but runs ~3.5us slower per rep from
SBUF port pressure, so fp16 is the default).

Each step is split into groups=2 independent column chains, issued
interleaved: chain A's op executes while chain B's previous op drains,
hiding the ~690ns serial-dependency stall of back-to-back dependent
custom ops (HW: 89.1 -> 79.7 us/rep; groups=4 regresses on dispatch
overhead + Act op count).

Spikes: Act engine Sign(m - 0.50012) -> u8 {0,1} via float->u8
saturation (sign=-1 saturates to 0); 0.5 < 0.50012 < 0.50049
(nextafter fp16 0.5) so the Sign input is never exactly 0 on the fp16
grid. Loads all on the sync HWDGE queue (routing any load via the
scalar queue stalls looped reps behind the prior rep's Sign ops),
spike stores on gpsimd SWDGE (scalar-queue stores measured 2.3x
worse; pair-batched DMA and deeper buffer pools neutral-to-worse).

HW decomposition (slope bench over hardware-loop reps, quiet machine):
compute-only chain 70.7 us/rep, full kernel 73-80 us/rep vs 102.9
baseline. Act busy ~61 us, DVE busy ~71 us incl. ~220ns/op residual;
DMA tax ~9 us. Measurements inflate up to +25% under noisy device
neighbors.
"""

import numpy as np

import concourse.bacc as bacc
import concourse.mybir as mybir
import concourse.tile as tile
from concourse.bass_utils import run_bass_kernel_spmd

N_CORES = 8
B, T, N = 64, 16, 65536
BC = B // N_CORES   # 8 batches per core
W = BC * N          # 524288 elements per timestep per core
F = W // 128        # 4096 free elements per partition
VTH = 0.5
SCL = 0.2           # f32 multiplier; oracle divides by 5.0 (<=1ulp apart)
PAIR_IO = False     # host-paired I/O layout (2 timesteps per DMA): no
                    # faster than per-step DMAs on HW (80.4 vs 79.7 us)

_OP_NAME = "LIF_STEP_ANT"
_nc_cache = None


def _register_lif_op():
    """Register the fused LIF-step DVE op in dve_ops.OPS (idempotent).

    out = select(in1 <= s0, in1 * s1, 0) + in0   [in0=x_t, in1=m_{t-1}]
    """
    import concourse.dve_ops as D
    from concourse.dve_spec import (
        C0,
        C1,
        Spec,
        Src0,
        Src1,
        Zero,
        _has_src1,
        lower,
        select,
    )
    from concourse.dve_uop import DveOpSpec

    for op in D.OPS:
        if op.name == _OP_NAME:
            return op

    def _ref(in0, in1, s0, s1, imm2):
        x = np.asarray(in0, np.float32)
        m = np.asarray(in1, np.float32)
        s0 = np.float32(s0)
        s1 = np.float32(s1)
        return np.where(m <= s0, m * s1, np.float32(0.0)) + x

    spec = Spec(body=select(Src1 <= C0, Src1 * C1, Zero) + Src0, reference=_ref)
    # sha depends only on the lowered uop bytes, not the opcode row, so it
    # can be pinned here before the op has its row assigned.
    shas = {
        ver: DveOpSpec(
            name=_OP_NAME, opcode=1, uops=lower(spec, ver=ver), rd1_en=_has_src1(spec)
        ).sha(ver)
        for ver in ("v3", "v4")
    }
    op = D.DveOp(_OP_NAME, spec, subdim=False, uops_sha=shas)
    D.OPS.append(op)
    D.CUSTOM_DVE_SPECS[op.name] = op.spec
    D._SUB_OPCODE_FOR_NAME[op.name] = D._CUSTOM_DVE_ROW_BASE + len(D.OPS) - 1
    assert max(D._SUB_OPCODE_FOR_NAME.values()) < 0x20
    return op


def _build(
    reps=1,
    internal_io=False,
    store_eng="gpsimd",  # SWDGE spike store (8% faster than Act HWDGE)
    xbufs=6,
    stbufs=3,
    sbufs=6,
    bodies=1,           # bench-only: bodies per For_i iteration
    head_split=False,   # t=1 load on the scalar HWDGE queue: saves ~2.9us
                        # single-shot but costs ~10us/rep looped (the load
                        # descriptor queues behind the prior rep's Sign ops)
    tail_chunks=4,      # column chunks for the last timestep
    unroll=1,           # sim-only: python-unrolled bodies (no For_i)
    no_store=False,     # bench-only: skip spike stores (DMA isolation)
    no_load=False,      # bench-only: skip x loads (DVE isolation)
    mem_f16=True,       # carry membrane in fp16 (less SBUF pressure)
    tchunk=1,           # timesteps per input load DMA (1 or 2)
    groups=2,           # independent column chains (interleaved DVE chunks)
    pair_io=PAIR_IO,    # host-paired layout: 2 timesteps per load/store DMA
    one_mtile=False,    # group chunks write slices of one m tile
):
    lif = _register_lif_op()
    f16 = mybir.dt.float16
    f32 = mybir.dt.float32
    u8 = mybir.dt.uint8
    act = mybir.ActivationFunctionType
    nc = bacc.Bacc("TRN2", target_bir_lowering=False, debug=False)
    xshape = [T // 2, 2 * W] if pair_io else [T, W]
    yshape = xshape
    if internal_io:
        # bench-only: stream against on-device DRAM so wall time is not
        # dominated by host<->device transfer of the real payload
        x = nc.dram_tensor("x_int", xshape, f16)
        y = nc.dram_tensor("y_int", yshape, u8)
        xin = nc.dram_tensor("x", [128, 16], f16, kind="ExternalInput")
        yout = nc.dram_tensor("y", [128, 16], f16, kind="ExternalOutput")
    else:
        x = nc.dram_tensor("x", xshape, f16, kind="ExternalInput")
        y = nc.dram_tensor("y", yshape, u8, kind="ExternalOutput")

    def dram_view(ap, t):
        return ap[t].rearrange("(p f) -> p f", p=128)

    def pair_view(ap, k):
        return ap[k].rearrange("(p two f) -> p two f", p=128, two=2)

    with tile.TileContext(nc) as tc:
        store = {"sync": nc.sync, "scalar": nc.scalar, "gpsimd": nc.gpsimd}[
            store_eng
        ]
        with (
            tc.tile_pool(name="xs", bufs=xbufs) as xp,
            tc.tile_pool(name="spk", bufs=sbufs) as sp,
            tc.tile_pool(name="state", bufs=stbufs) as st,
        ):
            bsp = st.tile([128, 1], f32, name="bsp", bufs=1)
            # f32 carry: threshold at nextafter(0.5) so the Sign input is
            # never exactly 0 (host-checked). fp16 carry: 0.50012 sits
            # strictly between the fp16 grid points 0.5 and 0.50049.
            nc.vector.memset(
                bsp[:],
                -0.50012 if mem_f16 else
                -float(np.nextafter(np.float32(0.5), np.float32(1))),
            )
            mdt = f16 if mem_f16 else f32

            def body(_i=None):
                mprev = None
                xt2 = None
                spk2 = None
                if no_load:
                    if pair_io:
                        xshared = xp.tile([128, 2, F], f16, tag="xt")
                        nc.sync.dma_start(xshared[:], pair_view(x.ap(), 0))
                    else:
                        xshared = xp.tile([128, F], f16, tag="xt")
                        nc.sync.dma_start(xshared[:], dram_view(x.ap(), 0))
                for t in range(T):
                    if pair_io:
                        if no_load:
                            xt2 = xshared
                        elif t % 2 == 0:
                            xt2 = xp.tile([128, 2, F], f16, tag="xt")
                            nc.sync.dma_start(xt2[:], pair_view(x.ap(), t // 2))
                        xt = xt2[:, t % 2]
                    elif no_load:
                        xt = xshared
                    elif head_split and t == 1:
                        # x1 on the scalar HWDGE queue, concurrent with x0 on
                        # sync: the t=1 fused op (which needs x0 AND x1)
                        # starts ~2.9us earlier than one serialized queue.
                        xt = xp.tile([128, F], f16, tag="xt")
                        nc.scalar.dma_start(xt[:], dram_view(x.ap(), t))
                    elif tchunk == 2 and t >= 2:
                        if t % 2 == 0:
                            xt2 = xp.tile([128, 2, F], f16, tag="xt")
                            nc.sync.dma_start(
                                xt2[:],
                                x.ap()[t : t + 2].rearrange(
                                    "t (p f) -> p t f", p=128
                                ),
                            )
                        xt = xt2[:, t % 2]
                    else:
                        xt = xp.tile([128, F], f16, tag="xt")
                        nc.sync.dma_start(xt[:], dram_view(x.ap(), t))
                    if pair_io:
                        if t % 2 == 0:
                            spk2 = sp.tile([128, 2, F], u8, tag="spk")
                        spk = spk2[:, t % 2]
                    else:
                        spk = sp.tile([128, F], u8, tag="spk")
                    last = t == T - 1
                    nchunk = max(tail_chunks, groups) if last else groups
                    fc = F // nchunk

                    def mview(a, b):
                        for tile, s0, w in mprev:
                            if a >= s0 and b <= s0 + w:
                                return tile[:, a - s0 : b - s0]
                        raise AssertionError((a, b))

                    mcur = []
                    mfull = None
                    if one_mtile and t > 0:
                        mfull = st.tile([128, F], mdt, tag="mem")
                    for c in range(nchunk):
                        a, b = c * fc, (c + 1) * fc
                        if t == 0:
                            if c == 0:
                                mcur = [(xt, 0, F)]
                            continue
                        if one_mtile:
                            mo = mfull[:, a:b]
                        else:
                            tag = f"mem{c}" if not last else f"memt{c}"
                            mt_ = st.tile([128, fc], mdt, tag=tag)
                            mo = mt_[:]
                        nc.vector._custom_dve(
                            lif, out=mo, in0=xt[:, a:b], in1=mview(a, b),
                            s0=VTH, s1=SCL,
                        )
                        if not one_mtile:
                            mcur.append((mt_, a, fc))
                    if one_mtile and t > 0:
                        mcur = [(mfull, 0, F)]
                    for mt, a, w in mcur:
                        mv = mt[:, a : a + w] if mt is xt else mt[:]
                        nc.scalar.activation(
                            spk[:, a : a + w], mv, act.Sign,
                            bias=bsp[:], scale=1.0,
                        )
                        if not no_store and last:
                            if pair_io:
                                store.dma_start(
                                    pair_view(y.ap(), t // 2)[:, :, a : a + w],
                                    spk2[:, :, a : a + w],
                                )
                            else:
                                store.dma_start(
                                    dram_view(y.ap(), t)[:, a : a + w],
                                    spk[:, a : a + w],
                                )
                    if not no_store and not last:
                        if pair_io:
                            if t % 2 == 1:
                                store.dma_start(
                                    pair_view(y.ap(), t // 2), spk2[:]
                                )
                        else:
                            store.dma_start(dram_view(y.ap(), t), spk[:])
                    mprev = mcur

            if internal_io:
                dummy = st.tile([128, 16], f16, tag="dummy")
                nc.sync.dma_start(dummy[:], xin.ap())
                nc.sync.dma_start(yout.ap(), dummy[:])
            if unroll > 1:
                assert reps == 1
                for _ in range(unroll):  # sim-only: loop-free steady state
                    body()
            elif reps == 1:
                body()
            else:
                assert reps % bodies == 0
                with tc.For_i(0, reps // bodies, 1) as i:
                    for _ in range(bodies):
                        body(i)
    nc.compile()
    return nc


def _get_nc():
    global _nc_cache
    if _nc_cache is None:
        _nc_cache = _build()
    return _nc_cache


def _shard(X):
    """[B, T, N] f32 -> per-core t-major fp16; pair-interleaved when PAIR_IO
    ([T/2, 2W] with x_{2k},x_{2k+1} contiguous per partition: one 16KB/
    partition burst loads two timesteps)."""
    Xh = X.astype(np.float16)
    shards = []
    for c in range(N_CORES):
        s = Xh[c * BC : (c + 1) * BC].transpose(1, 0, 2).reshape(T, W)
        if PAIR_IO:
            s = (
                s.reshape(T // 2, 2, 128, F)
                .transpose(0, 2, 1, 3)
                .reshape(T // 2, 2 * W)
            )
        shards.append(np.ascontiguousarray(s))
    return shards


def _unshard(parts):
    out = np.empty((B, T, N), dtype=np.float32)
    for c, p in enumerate(parts):
        if PAIR_IO:
            p = (
                p.reshape(T // 2, 128, 2, F)
                .transpose(0, 2, 1, 3)
                .reshape(T, W)
            )
        out[c * BC : (c + 1) * BC] = (
            p.reshape(T, BC, N).transpose(1, 0, 2).astype(np.float32)
        )
    return out


def _run(X, **spmd_kwargs):
    nc = _get_nc()
    in_maps = [{"x": s} for s in _shard(X)]
    res = run_bass_kernel_spmd(nc, in_maps, list(range(N_CORES)), **spmd_kwargs)
    out = _unshard([res.results[c]["y"] for c in range(N_CORES)])
    return out, res


def kernel(X):
    X = np.asarray(X, dtype=np.float32)
    out, _ = _run(X)
    return out
